# revision 1
# baseline (speedup 1.0000x reference)
"""Trainium2 Bass kernel for nn_ContrastiveLoss (circular-shift negatives).

Reference computation (B=4096, D=1024, S=5):
    d_p[k]      = ||v[k] - a[k] + eps||
    d_n1[k,m]   = ||v[k] - a[idx(k,m)] + eps||,  idx(k,m) = (k+m+1)%B  (m==k -> (k+1)%B)
    d_n2[k,m]   = ||a[k] - v[idx(k,m)] + eps||
    loss        = mean(relu(1 + 2*d_p - min_m d_n1 - min_m d_n2))

Strategy (8 cores, data-parallel over batch, 512 anchors/core + 5-row halo):
  - All distances via the norm expansion ||x-y||^2 = ||x||^2 + ||y||^2 - 2<x,y>.
    (The +eps inside the norm perturbs d^2 by ~1e-4 relative 2e-8 -> dropped;
     effect on the result is ~1e-6 relative, far below tolerance.)
  - <v[k], a[j]> for the band j in [k, k+5] plus row norms come from PE
    matmuls over transposed tiles: band1 = [V.A^T | V.V^T], band2 = [A.A^T | A.V^T]
    computed per 128-anchor block with a 2-group rhs access pattern (N=266)
    so float32r runs at 1 cycle/row.
  - Diagonal extraction: bands are bounced SBUF->DRAM, then strided DMA
    gathers (element stride 1066 = row pitch + 2) pull the 11 diagonals into
    lane-aligned [128, m] tiles. Small vector/scalar epilogue computes the
    hinge. Row norms ride along as the VV/AA diagonals of the same bands.
  - Anchors k<5 (where m==k rewrites the negative index) are recomputed
    exactly on the host in numpy and spliced in.
"""

import numpy as np

B, D, S = 4096, 1024, 5
NCORES = 8
SH = B // NCORES          # 512 anchors per core
ROWS = SH + S             # 517 rows needed per shard (incl. halo)
MARGIN = 1.0
EPS = 1e-6

_CACHE = {}


def _build():
    import concourse.bass as bass
    import concourse.bacc as bacc
    import concourse.tile as tile
    import concourse.mybir as mybir
    from concourse.masks import make_identity

    f32 = mybir.dt.float32
    f32r = mybir.dt.float32r

    nc = bacc.Bacc()
    v_ext = nc.declare_dram_parameter("v", [ROWS, D], f32, isOutput=False)
    a_ext = nc.declare_dram_parameter("a", [ROWS, D], f32, isOutput=False)
    loss_ext = nc.declare_dram_parameter("loss", [SH], f32, isOutput=True)

    NB = SH // 128            # 4 anchor blocks per core
    NC = D // 128             # 8 contraction chunks
    W = 520                   # column pitch of one tensor group in T_all
    BW = 133                  # band width per group (128 anchors + 5 halo)
    N2 = 2 * BW               # 266 = matmul moving free size (>=256 -> f32r fast)

    from contextlib import ExitStack

    with tile.TileContext(nc) as tc, ExitStack() as ctx:
        sing = ctx.enter_context(tc.tile_pool(name="sing", bufs=1))
        natp = ctx.enter_context(tc.tile_pool(name="natp", bufs=1))
        tallp = ctx.enter_context(tc.tile_pool(name="tall", bufs=1))
        tpsum = ctx.enter_context(tc.tile_pool(name="tpsum", bufs=5, space="PSUM"))
        hpsum = ctx.enter_context(tc.tile_pool(name="hpsum", bufs=1, space="PSUM"))
        bpsum = ctx.enter_context(tc.tile_pool(name="bpsum", bufs=2, space="PSUM"))
        dramp = ctx.enter_context(tc.tile_pool(name="dramp", bufs=2, space="DRAM"))
        ndram = ctx.enter_context(tc.tile_pool(name="ndram", bufs=1, space="DRAM"))
        ep = ctx.enter_context(tc.tile_pool(name="ep", bufs=1))

        identity = sing.tile([128, 128], f32, tag="ident")
        make_identity(nc, identity)

        # preload the ACT Sqrt table early so the epilogue doesn't stall on
        # a ~1.3us LoadActFuncSet.
        warm = sing.tile([128, 1], f32, tag="warm")
        nc.vector.memset(warm[:, :], 1.0)
        nc.scalar.activation(out=warm[:, :], in_=warm[:, :],
                             func=mybir.ActivationFunctionType.Sqrt)

        # T_all[p, c, 2j+s]: s=0 -> A^T col j, s=1 -> V^T col j (interleaved
        # so each block's 266-wide rhs window is one contiguous slice, as
        # required by the fp32r matmul ISA restrictions).
        t_all = tallp.tile([128, NC, 2 * W], f32r, tag="t_all")
        t_view = t_all[:, :, :].rearrange("p c (j s) -> p c j s", s=2)

        # ---- loads, transposes and bands, pipelined by 128-row group ----
        # Halo first: tiny and unblocks everything else early.
        halos = {}
        for ti, ext in ((0, a_ext), (1, v_ext)):
            halo = natp.tile([128, D], f32, tag=f"halo{ti}")
            (nc.sync if ti == 0 else nc.scalar).dma_start(out=halo[:5, :], in_=ext[512:ROWS, :])
            halos[ti] = halo

        # halo row norms (rows 512..516) directly from natural layout
        nvh = ep.tile([128, 1], f32, tag="nvh")
        nah = ep.tile([128, 1], f32, tag="nah")
        scr_h = ep.tile([128, D], f32, tag="scr_h")
        scr_h2 = ep.tile([128, D], f32, tag="scr_h2")
        A = mybir.AluOpType
        nc.scalar.activation(out=scr_h[:5, :], in_=halos[0][:5, :],
                             func=mybir.ActivationFunctionType.Square,
                             accum_out=nah[:5, :])
        nc.scalar.activation(out=scr_h2[:5, :], in_=halos[1][:5, :],
                             func=mybir.ActivationFunctionType.Square,
                             accum_out=nvh[:5, :])

        # halo transposes: rows 512..516 -> cols 512..516 of every chunk
        for ti in range(2):
            hps = hpsum.tile([128, 5 * NC], f32, tag="hps")
            for c in range(NC):
                nc.tensor.transpose(
                    hps[:, c * 5 : c * 5 + 5],
                    halos[ti][:5, c * 128 : (c + 1) * 128],
                    identity[:5, :5],
                )
            hview = hps[:, :].rearrange("p (c j) -> p c j", j=5)
            dsth = t_view[:, :, 512:517, ti]
            nc.vector.tensor_copy(dsth, hview)

        def band_matmul(lhs_j0, lhs_s, lhs_n, k0, tag):
            """PSUM [lhs_n, 266] = sum_c lhsT_c.T @ interleaved[A^T|V^T] window.

            lhsT = rows lhs_j0..lhs_j0+lhs_n of tensor lhs_s (0=A, 1=V);
            rhs  = contiguous cols 2*k0 .. 2*k0+265 (row-pairs k0..k0+132).
            Output col 2c+s = <lhs_row, (A if s==0 else V)[k0+c]>.
            """
            bp = bpsum.tile([128, N2], f32, tag="band")
            for c in range(NC):
                lhsT = t_view[:, c, lhs_j0 : lhs_j0 + lhs_n, lhs_s]
                rhs = t_all[:, c, 2 * k0 : 2 * k0 + N2]
                nc.tensor.matmul(bp[:lhs_n, :], lhsT, rhs, start=(c == 0), stop=(c == NC - 1))
            return bp

        b1acc = ep.tile([128, NB, N2], f32, tag="b1acc")
        b2acc = ep.tile([128, NB, N2], f32, tag="b2acc")
        bd1 = dramp.tile([128, NB, N2], f32, tag="bd1")
        bd2 = dramp.tile([128, NB, N2], f32, tag="bd2")
        # g1: j=0 dp dot, j=1 VV diag (nv), j=2,4,6,8,10 dn1 dots
        g1 = ep.tile([128, NB, 11], f32, tag="g1")
        # g2: j=0 AA diag (na), j=3,5,7,9,11 dn2 dots
        g2 = ep.tile([128, NB, 12], f32, tag="g2")
        nv_d = ndram.tile([ROWS + 3], f32, tag="nv_d")
        na_d = ndram.tile([ROWS + 3], f32, tag="na_d")

        def emit_band(b):
            k0 = 128 * b
            bp1 = band_matmul(k0, 1, 128, k0, f"b1_{b}")   # lhsT=V -> VA | VV
            bp2 = band_matmul(k0, 0, 128, k0, f"b2_{b}")   # lhsT=A -> AA | AV
            nc.vector.tensor_copy(b1acc[:, b, :], bp1[:, :])
            nc.scalar.copy(b2acc[:, b, :], bp2[:, :])

        def vec_ap(dram_tile, i0, dims):
            return bass.AP(tensor=dram_tile.tensor, offset=dram_tile.offset + i0, ap=dims)

        # bd layout flat(p, b, c) = 1064*p + 266*b + c; diagonal element
        # (p, b, j) of block b sits at c = 2p + j -> flat = 1066*p + 266*b + j.
        def block_gather(bdt, b, width):
            return bass.AP(
                tensor=bdt.tensor,
                offset=bdt.offset + 266 * b,
                ap=[[1066, 128], [1, width]],
            )

        def emit_extract(b, writeback=True):
            nc.sync.dma_start(out=bd1[:, b, :], in_=b1acc[:, b, :])
            nc.scalar.dma_start(out=bd2[:, b, :], in_=b2acc[:, b, :])
            nc.sync.dma_start(out=g1[:, b, :], in_=block_gather(bd1, b, 11))
            nc.scalar.dma_start(out=g2[:, b, :], in_=block_gather(bd2, b, 12))
            if writeback:
                # norms to the flat staging vectors, straight DRAM->DRAM
                # (VV diag at col 2p+1, AA diag at col 2p)
                nc.sync.dma_start(
                    out=vec_ap(nv_d, 128 * b, [[1, 128]]),
                    in_=bass.AP(tensor=bd1.tensor, offset=bd1.offset + 266 * b + 1,
                                ap=[[1066, 128]]),
                )
                nc.scalar.dma_start(
                    out=vec_ap(na_d, 128 * b, [[1, 128]]),
                    in_=bass.AP(tensor=bd2.tensor, offset=bd2.offset + 266 * b,
                                ap=[[1066, 128]]),
                )

        # halo norms land early (computed from natural tiles above)
        nc.sync.dma_start(out=vec_ap(nv_d, 512, [[1, 5]]), in_=nvh[:5, :])
        nc.scalar.dma_start(out=vec_ap(na_d, 512, [[1, 5]]), in_=nah[:5, :])

        # block-3 norms come from a direct ACT square+accum on the rg3 natural
        # tiles (cheap, lands mid-timeline) so the shifted gathers don't have
        # to wait for block 3's band extract.
        norma3 = ep.tile([128, 1], f32, tag="norma3")
        normv3 = ep.tile([128, 1], f32, tag="normv3")
        scr_n0 = ep.tile([128, D], f32, tag="scr_n0")
        scr_n1 = ep.tile([128, D], f32, tag="scr_n1")
        scr_n = [scr_n0, scr_n1]

        # Row-group loop: load rg -> transpose rg -> band (b = rg-1) as soon
        # as its inputs (row groups <= b+1) are in T_all.
        for rg in range(4):
            nats_rg = {}
            for ti, ext in ((0, a_ext), (1, v_ext)):
                nat = natp.tile([128, D], f32, tag=f"nat{ti}_{rg}")
                eng = nc.sync if (rg + ti) % 2 == 0 else nc.scalar
                eng.dma_start(out=nat[:, :], in_=ext[rg * 128 : (rg + 1) * 128, :])
                nats_rg[ti] = nat
                if rg == 3:
                    nacc = norma3 if ti == 0 else normv3
                    nc.scalar.activation(out=scr_n[ti][:, :], in_=nat[:, :],
                                         func=mybir.ActivationFunctionType.Square,
                                         accum_out=nacc[:, :])
                    eng.dma_start(out=vec_ap(nv_d if ti else na_d, 384, [[1, 128]]),
                                  in_=nacc[:, :])
            for ti in range(2):
                for half in range(2):
                    ps = tpsum.tile([128, 512], f32, tag="tps")
                    for ci in range(4):
                        c = half * 4 + ci
                        nc.tensor.transpose(
                            ps[:, ci * 128 : (ci + 1) * 128],
                            nats_rg[ti][:, c * 128 : (c + 1) * 128],
                            identity,
                        )
                    dst = t_view[:, half * 4 : half * 4 + 4, rg * 128 : (rg + 1) * 128, ti]
                    psv = ps[:, :].rearrange("p (c j) -> p c j", j=128)
                    if (ti + half) % 2 == 0:
                        nc.vector.tensor_copy(dst, psv)
                    else:
                        nc.scalar.copy(dst, psv)
            if rg >= 1:
                emit_band(rg - 1)
                emit_extract(rg - 1)
        # staging vector complete after extracts 0..2 + rg3 norms + halo:
        # fetch shifted views + per-row norm columns, overlapped with band 3
        nash = ep.tile([128, NB, 5], f32, tag="nash")  # ||a[k+1+m]||^2
        nvsh = ep.tile([128, NB, 5], f32, tag="nvsh")  # ||v[k+1+m]||^2
        nvcol = ep.tile([128, NB, 1], f32, tag="nvcol")
        nacol = ep.tile([128, NB, 1], f32, tag="nacol")
        nc.sync.dma_start(out=nash[:, :, :], in_=vec_ap(na_d, 1, [[1, 128], [128, NB], [1, 5]]))
        nc.scalar.dma_start(out=nvsh[:, :, :], in_=vec_ap(nv_d, 1, [[1, 128], [128, NB], [1, 5]]))
        nc.sync.dma_start(out=nvcol[:, :, :], in_=vec_ap(nv_d, 0, [[1, 128], [128, NB]]))
        nc.scalar.dma_start(out=nacol[:, :, :], in_=vec_ap(na_d, 0, [[1, 128], [128, NB]]))
        emit_band(3)
        emit_extract(3, writeback=False)

        # ---- epilogue (per block, so it overlaps remaining bands) ----
        dsq = ep.tile([128, NB, 11], f32, tag="dsq")
        dall = ep.tile([128, NB, 11], f32, tag="dall")
        dn1m = ep.tile([128, NB], f32, tag="dn1m")
        dn2m = ep.tile([128, NB], f32, tag="dn2m")
        tsum = ep.tile([128, NB], f32, tag="tsum")
        hpre = ep.tile([128, NB], f32, tag="hpre")
        lossn = ep.tile([128, NB], f32, tag="lossn")

        nv_t = nvcol[:, :, :]
        na_t = nacol[:, :, :]
        dn1_dots = g1[:, :, 1:11].rearrange("p b (j s) -> p b j s", s=2)[:, :, :, 1]
        dn2_dots = g2[:, :, 2:12].rearrange("p b (j s) -> p b j s", s=2)[:, :, :, 1]

        # shift bases precomputed as soon as nash/nvsh land (overlaps bands)
        base1 = ep.tile([128, NB, 5], f32, tag="base1")
        base2 = ep.tile([128, NB, 5], f32, tag="base2")
        nc.vector.tensor_add(base1[:, :, :], nash[:, :, :],
                             nv_t.broadcast_to([128, NB, 5]))
        nc.vector.tensor_add(base2[:, :, :], nvsh[:, :, :],
                             na_t.broadcast_to([128, NB, 5]))

        def epilogue(b):
            bs = slice(b, b + 1)
            # dn1^2 = -2*dot + (nv + na_shift);  dn2^2 = -2*dot + (na + nv_shift)
            nc.vector.tensor_scalar(out=dsq[:, bs, 0:5], in0=dn1_dots[:, bs, :],
                                    scalar1=-2.0, scalar2=None, op0=A.mult)
            nc.vector.tensor_add(dsq[:, bs, 0:5], dsq[:, bs, 0:5], base1[:, bs, :])
            nc.vector.tensor_scalar(out=dsq[:, bs, 5:10], in0=dn2_dots[:, bs, :],
                                    scalar1=-2.0, scalar2=None, op0=A.mult)
            nc.vector.tensor_add(dsq[:, bs, 5:10], dsq[:, bs, 5:10], base2[:, bs, :])
            # dp^2 = (-2*dot + nv) + na   (scalar2 is per-partition within a block)
            nc.vector.tensor_scalar(out=dsq[:, bs, 10:11], in0=g1[:, bs, 0:1],
                                    scalar1=-2.0, scalar2=nv_t[:, b, :],
                                    op0=A.mult, op1=A.add)
            nc.vector.tensor_add(dsq[:, bs, 10:11], dsq[:, bs, 10:11], na_t[:, bs, :])
            nc.scalar.activation(out=dall[:, bs, :], in_=dsq[:, bs, :],
                                 func=mybir.ActivationFunctionType.Sqrt)
            nc.vector.tensor_reduce(out=dn1m[:, bs], in_=dall[:, bs, 0:5],
                                    axis=mybir.AxisListType.X, op=A.min)
            nc.vector.tensor_reduce(out=dn2m[:, bs], in_=dall[:, bs, 5:10],
                                    axis=mybir.AxisListType.X, op=A.min)
            nc.vector.tensor_add(tsum[:, bs], dn1m[:, bs], dn2m[:, bs])
            nc.vector.tensor_scalar(out=hpre[:, bs], in0=dall[:, bs, 10], scalar1=2.0,
                                    scalar2=MARGIN, op0=A.mult, op1=A.add)
            nc.vector.tensor_sub(lossn[:, bs], hpre[:, bs], tsum[:, bs])
            nc.vector.tensor_scalar_max(out=lossn[:, bs], in0=lossn[:, bs], scalar1=0.0)
            eng = nc.sync if b % 2 == 0 else nc.scalar
            eng.dma_start(
                out=bass.AP(tensor=loss_ext, offset=128 * b, ap=[[1, 128]]),
                in_=lossn[:, bs],
            )

        for b in range(NB):
            epilogue(b)

    nc.finalize()
    return nc


def _exact_losses_head(vfeat, afeat, ks):
    """Exact reference loss for anchors in ks (handles the m==k index rewrite)."""
    v = vfeat.astype(np.float64)
    a = afeat.astype(np.float64)
    out = []
    for k in ks:
        idx = [(m + k + 1) % B if m != k else (k + 1) % B for m in range(S)]
        d_p = np.sqrt(np.sum((v[k] - a[k] + EPS) ** 2))
        d1 = min(np.sqrt(np.sum((v[k] - a[j] + EPS) ** 2)) for j in idx)
        d2 = min(np.sqrt(np.sum((a[k] - v[j] + EPS) ** 2)) for j in idx)
        out.append(max(MARGIN + 2.0 * d_p - d1 - d2, 0.0))
    return out


def run_kernel(vfeat, afeat, trace=False):
    from concourse.bass_utils import run_bass_kernel_spmd

    vfeat = np.ascontiguousarray(np.asarray(vfeat, dtype=np.float32))
    afeat = np.ascontiguousarray(np.asarray(afeat, dtype=np.float32))

    if "nc" not in _CACHE:
        _CACHE["nc"] = _build()
    nc = _CACHE["nc"]

    in_maps = []
    for c in range(NCORES):
        lo = c * SH
        idx = np.arange(lo, lo + ROWS) % B
        in_maps.append({"v": vfeat[idx], "a": afeat[idx]})

    res = run_bass_kernel_spmd(nc, in_maps, core_ids=list(range(NCORES)), trace=trace)
    losses = np.concatenate([res.results[c]["loss"] for c in range(NCORES)])

    total = float(np.sum(losses[S:], dtype=np.float64))
    total += sum(_exact_losses_head(vfeat, afeat, range(S)))
    mean = np.float32(total / B)
    return np.asarray(mean, dtype=np.float32), res


def kernel(vfeat, afeat):
    out, _ = run_kernel(vfeat, afeat, trace=False)
    return out



# revision 12
# speedup vs baseline: 1.4030x; 1.4030x over previous
"""Trainium2 Bass kernel for nn_ContrastiveLoss (circular-shift negatives).

Reference computation (B=4096, D=1024, S=5):
    d_p[k]      = ||v[k] - a[k] + eps||
    d_n1[k,m]   = ||v[k] - a[idx(k,m)] + eps||,  idx(k,m) = (k+m+1)%B  (m==k -> (k+1)%B)
    d_n2[k,m]   = ||a[k] - v[idx(k,m)] + eps||
    loss        = mean(relu(1 + 2*d_p - min_m d_n1 - min_m d_n2))

Strategy (8 cores, data-parallel over batch, 512 anchors/core + 5-row halo):
  - All distances via ||x-y||^2 = ||x||^2 + ||y||^2 - 2<x,y> (the +eps term
    is dropped; its effect is ~1e-6 relative, far below tolerance).
  - The host pre-transposes and interleaves the shard into PE-ready layout
    t[p, c, 2w+s] = (A if s==0 else V)[w, 128c+p], so no on-device transposes
    are needed and the DMA loads land directly in matmul operand layout.
  - Loads are split into 4 column-group DMAs sized so that anchor block b's
    band matmuls unblock as soon as group b lands (load/compute pipeline).
  - Per 128-anchor block, two 266-wide f32r band matmuls (1 cycle/row):
    band1 = [V.A^T | V.V^T], band2 = [A.A^T | A.V^T] interleaved windows.
    Bands accumulate in PSUM and are DMA'd straight to a DRAM scratch.
  - One global strided gather (element stride 268 = row pitch + 2) shears the
    11 needed diagonals of every band into lane-aligned SBUF: d_p dot, dn1
    dots, dn2 dots, and the row norms (VV/AA diagonals) per anchor.
  - Shifted norms ||x[k+m+1]||^2 are produced on-chip by 10 tiny shifted-
    identity matmuls (partition shift), with halo-row norms from an ACT
    square+accum over the 5 natural halo rows (tiny extra input).
  - Small vector/scalar/pool epilogue computes sqrt, mins and the hinge.
  - Anchors k<5 (where m==k rewrites the negative index) are recomputed
    exactly on the host in numpy and spliced in; the mean is a host reduce.
"""

import numpy as np

B, D, S = 4096, 1024, 5
NCORES = 8
SH = B // NCORES          # 512 anchors per core
ROWS = SH + S             # 517 rows per shard (incl. halo)
NB = SH // 128            # 4 anchor blocks
NC = D // 128             # 8 contraction chunks
TW = 2 * ROWS             # 1034 interleaved t columns
MARGIN = 1.0
EPS = 1e-6

_CACHE = {}


def _build():
    import concourse.bass as bass
    import concourse.bacc as bacc
    import concourse.tile as tile
    import concourse.mybir as mybir

    from contextlib import ExitStack

    f32 = mybir.dt.float32
    f32r = mybir.dt.float32r
    A = mybir.AluOpType

    nc = bacc.Bacc()
    t_ext = nc.declare_dram_parameter("t", [128, NC, TW], f32r, isOutput=False)
    hn_ext = nc.declare_dram_parameter("hn", [2, S, D], f32, isOutput=False)
    loss_ext = nc.declare_dram_parameter("loss", [SH], f32, isOutput=True)

    with tile.TileContext(nc) as tc, ExitStack() as ctx:
        sing = ctx.enter_context(tc.tile_pool(name="sing", bufs=1))
        tp = ctx.enter_context(tc.tile_pool(name="tp", bufs=1))
        ep = ctx.enter_context(tc.tile_pool(name="ep", bufs=1))
        bandp = ctx.enter_context(tc.tile_pool(name="bandp", bufs=3, space="PSUM"))
        bsb = ctx.enter_context(tc.tile_pool(name="bsb", bufs=2))
        shp = ctx.enter_context(tc.tile_pool(name="shp", bufs=1, space="PSUM"))
        dramp = ctx.enter_context(tc.tile_pool(name="dramp", bufs=1, space="DRAM"))

        # preload the ACT Sqrt table early so the epilogue doesn't stall on
        # a ~1.3us LoadActFuncSet.
        warm = sing.tile([128, 1], f32, tag="warm")
        nc.vector.memset(warm[:, :], 1.0)
        nc.scalar.activation(out=warm[:, :], in_=warm[:, :],
                             func=mybir.ActivationFunctionType.Sqrt)

        # partition-shift matrices for the shifted-norm matmuls (see below):
        # Mpack[:, j] = shift-down-by-(j+1): M[i, p] = 1 iff i == p + j + 1
        # Epack[:, j] = wrap rows:           E[i, p] = 1 iff p == 128-(j+1)+i
        ones = sing.tile([128, 128], f32, tag="ones")
        nc.vector.memset(ones[:, :], 1.0)
        mpack = sing.tile([128, S, 128], f32, tag="mpack")
        epack = sing.tile([128, S, 128], f32, tag="epack")
        for j in range(1, S + 1):
            nc.gpsimd.affine_select(out=mpack[:, j - 1, :], in_=ones[:, :],
                                    pattern=[[1, 128]], base=j,
                                    channel_multiplier=-1,
                                    compare_op=A.is_equal, fill=0.0)
            nc.gpsimd.affine_select(out=epack[0:S, j - 1, :], in_=ones[0:S, :],
                                    pattern=[[1, 128]], base=-(128 - j),
                                    channel_multiplier=-1,
                                    compare_op=A.is_equal, fill=0.0)

        tsb = tp.tile([128, NC, TW], f32r, tag="tsb")
        tv = tsb[:, :, :].rearrange("p c (w s) -> p c w s", s=2)  # s: 0=A, 1=V
        hn_sb = tp.tile([128, 2, D], f32, tag="hn")
        g = ep.tile([128, 96], f32, tag="g")
        gm = g[:, :].rearrange("p (t b m s) -> p t b m s", t=2, b=NB, m=6, s=2)
        Nrm = ep.tile([128, 2, NB + 1], f32, tag="nrm")   # [p, {nv,na}, block(+halo)]
        scr = ep.tile([128, D], f32, tag="scr")

        # DRAM band scratch: flat(t, b, p, c) = 136192*t + 34048*b + 266*p + c
        band_d = dramp.tile([2 * NB * 128 * 266], f32, tag="band_d")

        # ---- input loads: 4 column groups; group b unblocks anchor block b
        GRP = [(0, 266), (266, 532), (532, 798), (798, TW)]
        for g0, g1 in GRP:
            nc.sync.dma_start(out=tsb[:, :, g0:g1], in_=t_ext[:, :, g0:g1])
        # tiny natural-layout halo rows (for halo norms only), last: off path
        nc.sync.dma_start(
            out=hn_sb[0:S, :, :],
            in_=bass.AP(tensor=hn_ext, offset=0, ap=[[D, S], [S * D, 2], [1, D]]),
        )

        # halo norms -> Nrm[:, :, NB] (col 4). hn[:,0]=A rows -> na, hn[:,1]=V -> nv
        nc.scalar.activation(out=scr[0:S, :], in_=hn_sb[0:S, 0, :],
                             func=mybir.ActivationFunctionType.Square,
                             accum_out=Nrm[0:S, 1, NB:NB + 1])
        nc.scalar.activation(out=scr[0:S, :], in_=hn_sb[0:S, 1, :],
                             func=mybir.ActivationFunctionType.Square,
                             accum_out=Nrm[0:S, 0, NB:NB + 1])

        # ---- bands + PSUM->DRAM writes, per block
        for b in range(NB):
            k0 = 128 * b
            bp = bandp.tile([128, 2, 512], f32, tag="bp")
            for c in range(NC):
                nc.tensor.matmul(bp[:, 0, 0:266], tv[:, c, k0:k0 + 128, 1],
                                 tsb[:, c, 2 * k0:2 * k0 + 266],
                                 start=(c == 0), stop=(c == NC - 1))
            for c in range(NC):
                nc.tensor.matmul(bp[:, 1, 0:266], tv[:, c, k0:k0 + 128, 0],
                                 tsb[:, c, 2 * k0:2 * k0 + 266],
                                 start=(c == 0), stop=(c == NC - 1))
            bs = bsb.tile([128, 2, 266], f32, tag="bs")
            if b % 2 == 0:
                nc.vector.tensor_copy(bs[:, :, :], bp[:, :, 0:266])
            else:
                nc.scalar.copy(bs[:, :, :], bp[:, :, 0:266])
            nc.sync.dma_start(
                out=bass.AP(tensor=band_d.tensor, offset=band_d.offset + 34048 * b,
                            ap=[[266, 128], [136192, 2], [1, 266]]),
                in_=bs[:, :, :],
            )

        # ---- global diagonal shear-gather: g[p, t, b, j] = band[t,b][p, 2p+j]
        nc.sync.dma_start(
            out=g[:, :],
            in_=bass.AP(tensor=band_d.tensor, offset=band_d.offset,
                        ap=[[268, 128], [136192, 2], [34048, NB], [1, 12]]),
        )

        # per-block norms from the gathered VV/AA diagonals
        nc.vector.tensor_copy(Nrm[:, 0, 0:NB], gm[:, 0, :, 0, 1])  # nv
        nc.vector.tensor_copy(Nrm[:, 1, 0:NB], gm[:, 1, :, 0, 0])  # na

        # ---- shifted norms via partition-shift matmuls: P[p, m, t, b] = N[p+m+1, t, b]
        psh = shp.tile([128, S, 2, NB], f32, tag="psh")
        for j in range(1, S + 1):
            m = j - 1
            nc.tensor.matmul(psh[:, m, :, :], mpack[:, m, :],
                             Nrm[:, :, 0:NB], start=True, stop=False)
            nc.tensor.matmul(psh[:, m, :, :], epack[0:j, m, :],
                             Nrm[0:j, :, 1:NB + 1], start=False, stop=True)
        psb = ep.tile([128, S, 2, NB], f32, tag="psb")
        nc.vector.tensor_copy(psb[:, :, :, :], psh[:, :, :, :])
        pv = psb[:, :, :, :].rearrange("p m t b -> p t b m")

        # ---- epilogue
        dn1_dots = gm[:, 0, :, 1:6, 0]   # <v[k], a[k+m+1]>
        dn2_dots = gm[:, 1, :, 1:6, 1]   # <a[k], v[k+m+1]>
        dp_dot = gm[:, 0, :, 0:1, 0]
        nv_col = gm[:, 0, :, 0:1, 1]
        na_col = gm[:, 1, :, 0:1, 0]

        base1 = ep.tile([128, NB, S], f32, tag="base1")
        base2 = ep.tile([128, NB, S], f32, tag="base2")
        dsq = ep.tile([128, NB, 11], f32, tag="dsq")
        dall = ep.tile([128, NB, 11], f32, tag="dall")
        dn1m = ep.tile([128, NB], f32, tag="dn1m")
        dn2m = ep.tile([128, NB], f32, tag="dn2m")
        tsum = ep.tile([128, NB], f32, tag="tsum")
        lossn = ep.tile([128, NB], f32, tag="lossn")

        # dn1^2 = -2*dot + (nv[k] + na[k+m+1]);  dn2^2 = -2*dot + (na[k] + nv[k+m+1])
        nc.vector.tensor_add(base1[:, :, :], pv[:, 1, :, :],
                             nv_col.broadcast_to([128, NB, S]))
        nc.vector.scalar_tensor_tensor(out=dsq[:, :, 0:5], in0=dn1_dots, scalar=-2.0,
                                       in1=base1[:, :, :], op0=A.mult, op1=A.add)
        nc.gpsimd.tensor_add(base2[:, :, :], pv[:, 0, :, :],
                             na_col.broadcast_to([128, NB, S]))
        nc.vector.scalar_tensor_tensor(out=dsq[:, :, 5:10], in0=dn2_dots, scalar=-2.0,
                                       in1=base2[:, :, :], op0=A.mult, op1=A.add)
        nc.vector.scalar_tensor_tensor(out=dsq[:, :, 10:11], in0=dp_dot, scalar=-2.0,
                                       in1=nv_col, op0=A.mult, op1=A.add)
        nc.vector.tensor_add(dsq[:, :, 10:11], dsq[:, :, 10:11], na_col)

        nc.scalar.activation(out=dall[:, :, :], in_=dsq[:, :, :],
                             func=mybir.ActivationFunctionType.Sqrt)
        nc.vector.tensor_reduce(out=dn1m[:, :], in_=dall[:, :, 0:5],
                                axis=mybir.AxisListType.X, op=A.min)
        nc.vector.tensor_reduce(out=dn2m[:, :], in_=dall[:, :, 5:10],
                                axis=mybir.AxisListType.X, op=A.min)
        nc.vector.tensor_add(tsum[:, :], dn1m[:, :], dn2m[:, :])
        nc.vector.scalar_tensor_tensor(out=lossn[:, :], in0=dall[:, :, 10], scalar=2.0,
                                       in1=tsum[:, :], op0=A.mult, op1=A.subtract)
        nc.vector.tensor_scalar(out=lossn[:, :], in0=lossn[:, :], scalar1=MARGIN,
                                scalar2=0.0, op0=A.add, op1=A.max)

        nc.sync.dma_start(
            out=bass.AP(tensor=loss_ext, offset=0, ap=[[1, 128], [128, NB]]),
            in_=lossn[:, :],
        )

    nc.finalize()
    return nc


def _exact_losses_head(vfeat, afeat, ks):
    """Exact reference loss for anchors in ks (handles the m==k index rewrite)."""
    v = vfeat.astype(np.float64)
    a = afeat.astype(np.float64)
    out = []
    for k in ks:
        idx = [(m + k + 1) % B if m != k else (k + 1) % B for m in range(S)]
        d_p = np.sqrt(np.sum((v[k] - a[k] + EPS) ** 2))
        d1 = min(np.sqrt(np.sum((v[k] - a[j] + EPS) ** 2)) for j in idx)
        d2 = min(np.sqrt(np.sum((a[k] - v[j] + EPS) ** 2)) for j in idx)
        out.append(max(MARGIN + 2.0 * d_p - d1 - d2, 0.0))
    return out


def run_kernel(vfeat, afeat, trace=False):
    from concourse.bass_utils import run_bass_kernel_spmd

    vfeat = np.ascontiguousarray(np.asarray(vfeat, dtype=np.float32))
    afeat = np.ascontiguousarray(np.asarray(afeat, dtype=np.float32))

    if "nc" not in _CACHE:
        _CACHE["nc"] = _build()
    nc = _CACHE["nc"]

    in_maps = []
    for c in range(NCORES):
        lo = c * SH
        rows = np.arange(lo, lo + ROWS) % B
        Ar, Vr = afeat[rows], vfeat[rows]                 # [517, 1024]
        St = np.stack([Ar, Vr], axis=-1)                  # [w, d, s]
        t = np.ascontiguousarray(
            St.reshape(ROWS, NC, 128, 2).transpose(2, 1, 0, 3).reshape(128, NC, TW)
        )
        hn = np.ascontiguousarray(np.stack([Ar[SH:], Vr[SH:]], axis=0))
        in_maps.append({"t": t, "hn": hn})

    res = run_bass_kernel_spmd(nc, in_maps, core_ids=list(range(NCORES)), trace=trace)
    losses = np.concatenate([res.results[c]["loss"] for c in range(NCORES)])

    total = float(np.sum(losses[S:], dtype=np.float64))
    total += sum(_exact_losses_head(vfeat, afeat, range(S)))
    mean = np.float32(total / B)
    return np.asarray(mean, dtype=np.float32), res


def kernel(vfeat, afeat):
    out, _ = run_kernel(vfeat, afeat, trace=False)
    return out


# revision 18
# speedup vs baseline: 1.5857x; 1.1302x over previous
"""Trainium2 Bass kernel for nn_ContrastiveLoss (circular-shift negatives).

Reference computation (B=4096, D=1024, S=5):
    d_p[k]      = ||v[k] - a[k] + eps||
    d_n1[k,m]   = ||v[k] - a[idx(k,m)] + eps||,  idx(k,m) = (k+m+1)%B  (m==k -> (k+1)%B)
    d_n2[k,m]   = ||a[k] - v[idx(k,m)] + eps||
    loss        = mean(relu(1 + 2*d_p - min_m d_n1 - min_m d_n2))

Strategy (8 cores, data-parallel over batch, 512 anchors/core + 5-row halo):
  - All distances via ||x-y||^2 = ||x||^2 + ||y||^2 - 2<x,y> (the +eps term
    is dropped; its effect is ~1e-6 relative, far below tolerance).
  - The host pre-transposes and interleaves the shard into PE-ready layout
    t[p, c, 2w+s] = (A if s==0 else V)[w, 128c+p], so no on-device transposes
    are needed and the DMA loads land directly in matmul operand layout.
  - Loads are split into 4 column-group DMAs sized so that anchor block b's
    band matmuls unblock as soon as group b lands (load/compute pipeline).
  - Per 128-anchor block, two 266-wide f32r band matmuls (1 cycle/row):
    band1 = [V.A^T | V.V^T], band2 = [A.A^T | A.V^T] interleaved windows.
    Bands accumulate in PSUM and are DMA'd straight to a DRAM scratch.
  - One global strided gather (element stride 268 = row pitch + 2) shears the
    11 needed diagonals of every band into lane-aligned SBUF: d_p dot, dn1
    dots, dn2 dots, and the row norms (VV/AA diagonals) per anchor.
  - Shifted norms ||x[k+m+1]||^2 are produced on-chip by 10 tiny shifted-
    identity matmuls (partition shift), with halo-row norms from an ACT
    square+accum over the 5 natural halo rows (tiny extra input).
  - Small vector/scalar/pool epilogue computes sqrt, mins and the hinge.
  - Anchors k<5 (where m==k rewrites the negative index) are recomputed
    exactly on the host in numpy and spliced in; the mean is a host reduce.
"""

import numpy as np

B, D, S = 4096, 1024, 5
NCORES = 8
SH = B // NCORES          # 512 anchors per core
ROWS = SH + S             # 517 rows per shard (incl. halo)
NB = SH // 128            # 4 anchor blocks
NC = D // 128             # 8 contraction chunks
TW = 2 * ROWS             # 1034 interleaved t columns
MARGIN = 1.0
EPS = 1e-6

_CACHE = {}


def _build():
    import concourse.bass as bass
    import concourse.bacc as bacc
    import concourse.tile as tile
    import concourse.mybir as mybir

    from contextlib import ExitStack

    f32 = mybir.dt.float32
    f32r = mybir.dt.float32r
    A = mybir.AluOpType

    nc = bacc.Bacc()
    t_ext = nc.declare_dram_parameter("t", [128, TW, NC], f32r, isOutput=False)
    hn_ext = nc.declare_dram_parameter("hn", [2, S, D], f32, isOutput=False)
    loss_ext = nc.declare_dram_parameter("loss", [SH], f32, isOutput=True)

    with tile.TileContext(nc) as tc, ExitStack() as ctx:
        sing = ctx.enter_context(tc.tile_pool(name="sing", bufs=1))
        tp = ctx.enter_context(tc.tile_pool(name="tp", bufs=1))
        ep = ctx.enter_context(tc.tile_pool(name="ep", bufs=1))
        bandp = ctx.enter_context(tc.tile_pool(name="bandp", bufs=3, space="PSUM"))
        bsb = ctx.enter_context(tc.tile_pool(name="bsb", bufs=4))
        shp = ctx.enter_context(tc.tile_pool(name="shp", bufs=1, space="PSUM"))
        dramp = ctx.enter_context(tc.tile_pool(name="dramp", bufs=1, space="DRAM"))

        # preload the ACT Sqrt table early so the epilogue doesn't stall on
        # a ~1.3us LoadActFuncSet.
        warm = sing.tile([128, 1], f32, tag="warm")
        nc.vector.memset(warm[:, :], 1.0)
        nc.scalar.activation(out=warm[:, :], in_=warm[:, :],
                             func=mybir.ActivationFunctionType.Sqrt)

        # partition-shift matrices for the shifted-norm matmuls (see below):
        # Mpack[:, j] = shift-down-by-(j+1): M[i, p] = 1 iff i == p + j + 1
        # Epack[:, j] = wrap rows:           E[i, p] = 1 iff p == 128-(j+1)+i
        ones = sing.tile([128, 128], f32, tag="ones")
        nc.vector.memset(ones[:, :], 1.0)
        mpack = sing.tile([128, S, 128], f32, tag="mpack")
        epack = sing.tile([128, S, 128], f32, tag="epack")
        for j in range(1, S + 1):
            nc.gpsimd.affine_select(out=mpack[:, j - 1, :], in_=ones[:, :],
                                    pattern=[[1, 128]], base=j,
                                    channel_multiplier=-1,
                                    compare_op=A.is_equal, fill=0.0)
            nc.gpsimd.affine_select(out=epack[0:S, j - 1, :], in_=ones[0:S, :],
                                    pattern=[[1, 128]], base=-(128 - j),
                                    channel_multiplier=-1,
                                    compare_op=A.is_equal, fill=0.0)

        # column-major [p, col, chunk] so the 4 group loads cover disjoint
        # flat ranges (Tile's interval dep tracking stays precise).
        tsb = tp.tile([128, TW, NC], f32r, tag="tsb")
        tv = tsb[:, :, :].rearrange("p (w s) c -> p w s c", s=2)  # s: 0=A, 1=V
        hn_sb = tp.tile([128, 2, D], f32, tag="hn")
        g = ep.tile([128, 96], f32, tag="g")
        gm = g[:, :].rearrange("p (t b m s) -> p t b m s", t=2, b=NB, m=6, s=2)
        Nrm = ep.tile([128, 2, NB + 1], f32, tag="nrm")   # [p, {nv,na}, block(+halo)]
        scr = ep.tile([128, D], f32, tag="scr")

        # DRAM band scratch: flat(t, b, p, c) = 136192*t + 34048*b + 266*p + c
        band_d = dramp.tile([2 * NB * 128 * 266], f32, tag="band_d")

        # ---- input loads: 4 column groups; group b unblocks anchor block b
        GRP = [(0, 266), (266, 532), (532, 798), (798, TW)]
        for g0, g1 in GRP:
            nc.sync.dma_start(out=tsb[:, g0:g1, :], in_=t_ext[:, g0:g1, :])
        # tiny natural-layout halo rows (for halo norms only), last: off path
        nc.sync.dma_start(
            out=hn_sb[0:S, :, :],
            in_=bass.AP(tensor=hn_ext, offset=0, ap=[[D, S], [S * D, 2], [1, D]]),
        )

        # halo norms -> Nrm[:, :, NB] (col 4). hn[:,0]=A rows -> na, hn[:,1]=V -> nv
        nc.scalar.activation(out=scr[0:S, :], in_=hn_sb[0:S, 0, :],
                             func=mybir.ActivationFunctionType.Square,
                             accum_out=Nrm[0:S, 1, NB:NB + 1])
        nc.scalar.activation(out=scr[0:S, :], in_=hn_sb[0:S, 1, :],
                             func=mybir.ActivationFunctionType.Square,
                             accum_out=Nrm[0:S, 0, NB:NB + 1])

        # ---- bands + PSUM->DRAM writes, per block
        for b in range(NB):
            k0 = 128 * b
            bp = bandp.tile([128, 2, 512], f32, tag="bp")
            for c in range(NC):
                nc.tensor.matmul(bp[:, 0, 0:266], tv[:, k0:k0 + 128, 1, c],
                                 tsb[:, 2 * k0:2 * k0 + 266, c],
                                 start=(c == 0), stop=(c == NC - 1))
            for c in range(NC):
                nc.tensor.matmul(bp[:, 1, 0:266], tv[:, k0:k0 + 128, 0, c],
                                 tsb[:, 2 * k0:2 * k0 + 266, c],
                                 start=(c == 0), stop=(c == NC - 1))
            bs = bsb.tile([128, 2, 266], f32, tag="bs")
            if b % 2 == 0:
                nc.vector.tensor_copy(bs[:, :, :], bp[:, :, 0:266])
            else:
                nc.scalar.copy(bs[:, :, :], bp[:, :, 0:266])
            nc.sync.dma_start(
                out=bass.AP(tensor=band_d.tensor, offset=band_d.offset + 34048 * b,
                            ap=[[266, 128], [136192, 2], [1, 266]]),
                in_=bs[:, :, :],
            )

        # ---- global diagonal shear-gather: g[p, t, b, j] = band[t,b][p, 2p+j]
        nc.sync.dma_start(
            out=g[:, :],
            in_=bass.AP(tensor=band_d.tensor, offset=band_d.offset,
                        ap=[[268, 128], [136192, 2], [34048, NB], [1, 12]]),
        )

        # per-block norms from the gathered VV/AA diagonals
        nc.vector.tensor_copy(Nrm[:, 0, 0:NB], gm[:, 0, :, 0, 1])  # nv
        nc.vector.tensor_copy(Nrm[:, 1, 0:NB], gm[:, 1, :, 0, 0])  # na

        # ---- shifted norms via partition-shift matmuls: P[p, m, t, b] = N[p+m+1, t, b]
        psh = shp.tile([128, S, 2, NB], f32, tag="psh")
        for j in range(1, S + 1):
            m = j - 1
            nc.tensor.matmul(psh[:, m, :, :], mpack[:, m, :],
                             Nrm[:, :, 0:NB], start=True, stop=False)
            nc.tensor.matmul(psh[:, m, :, :], epack[0:j, m, :],
                             Nrm[0:j, :, 1:NB + 1], start=False, stop=True)
        psb = ep.tile([128, S, 2, NB], f32, tag="psb")
        nc.vector.tensor_copy(psb[:, :, :, :], psh[:, :, :, :])
        pv = psb[:, :, :, :].rearrange("p m t b -> p t b m")

        # ---- epilogue
        dn1_dots = gm[:, 0, :, 1:6, 0]   # <v[k], a[k+m+1]>
        dn2_dots = gm[:, 1, :, 1:6, 1]   # <a[k], v[k+m+1]>
        dp_dot = gm[:, 0, :, 0:1, 0]
        nv_col = gm[:, 0, :, 0:1, 1]
        na_col = gm[:, 1, :, 0:1, 0]

        base1 = ep.tile([128, NB, S], f32, tag="base1")
        base2 = ep.tile([128, NB, S], f32, tag="base2")
        dsq = ep.tile([128, NB, 11], f32, tag="dsq")
        dall = ep.tile([128, NB, 11], f32, tag="dall")
        dn1m = ep.tile([128, NB], f32, tag="dn1m")
        dn2m = ep.tile([128, NB], f32, tag="dn2m")
        tsum = ep.tile([128, NB], f32, tag="tsum")
        lossn = ep.tile([128, NB], f32, tag="lossn")

        # dn1^2 = -2*dot + (nv[k] + na[k+m+1]);  dn2^2 = -2*dot + (na[k] + nv[k+m+1])
        nc.vector.tensor_add(base1[:, :, :], pv[:, 1, :, :],
                             nv_col.broadcast_to([128, NB, S]))
        nc.vector.scalar_tensor_tensor(out=dsq[:, :, 0:5], in0=dn1_dots, scalar=-2.0,
                                       in1=base1[:, :, :], op0=A.mult, op1=A.add)
        nc.gpsimd.tensor_add(base2[:, :, :], pv[:, 0, :, :],
                             na_col.broadcast_to([128, NB, S]))
        nc.vector.scalar_tensor_tensor(out=dsq[:, :, 5:10], in0=dn2_dots, scalar=-2.0,
                                       in1=base2[:, :, :], op0=A.mult, op1=A.add)
        nc.vector.scalar_tensor_tensor(out=dsq[:, :, 10:11], in0=dp_dot, scalar=-2.0,
                                       in1=nv_col, op0=A.mult, op1=A.add)
        nc.vector.tensor_add(dsq[:, :, 10:11], dsq[:, :, 10:11], na_col)

        nc.scalar.activation(out=dall[:, :, :], in_=dsq[:, :, :],
                             func=mybir.ActivationFunctionType.Sqrt)
        nc.vector.tensor_reduce(out=dn1m[:, :], in_=dall[:, :, 0:5],
                                axis=mybir.AxisListType.X, op=A.min)
        nc.vector.tensor_reduce(out=dn2m[:, :], in_=dall[:, :, 5:10],
                                axis=mybir.AxisListType.X, op=A.min)
        nc.vector.tensor_add(tsum[:, :], dn1m[:, :], dn2m[:, :])
        nc.vector.scalar_tensor_tensor(out=lossn[:, :], in0=dall[:, :, 10], scalar=2.0,
                                       in1=tsum[:, :], op0=A.mult, op1=A.subtract)
        nc.vector.tensor_scalar(out=lossn[:, :], in0=lossn[:, :], scalar1=MARGIN,
                                scalar2=0.0, op0=A.add, op1=A.max)

        nc.sync.dma_start(
            out=bass.AP(tensor=loss_ext, offset=0, ap=[[1, 128], [128, NB]]),
            in_=lossn[:, :],
        )

    nc.finalize()
    return nc


def _exact_losses_head(vfeat, afeat, ks):
    """Exact reference loss for anchors in ks (handles the m==k index rewrite)."""
    v = vfeat.astype(np.float64)
    a = afeat.astype(np.float64)
    out = []
    for k in ks:
        idx = [(m + k + 1) % B if m != k else (k + 1) % B for m in range(S)]
        d_p = np.sqrt(np.sum((v[k] - a[k] + EPS) ** 2))
        d1 = min(np.sqrt(np.sum((v[k] - a[j] + EPS) ** 2)) for j in idx)
        d2 = min(np.sqrt(np.sum((a[k] - v[j] + EPS) ** 2)) for j in idx)
        out.append(max(MARGIN + 2.0 * d_p - d1 - d2, 0.0))
    return out


def run_kernel(vfeat, afeat, trace=False):
    from concourse.bass_utils import run_bass_kernel_spmd

    vfeat = np.ascontiguousarray(np.asarray(vfeat, dtype=np.float32))
    afeat = np.ascontiguousarray(np.asarray(afeat, dtype=np.float32))

    if "nc" not in _CACHE:
        _CACHE["nc"] = _build()
    nc = _CACHE["nc"]

    in_maps = []
    for c in range(NCORES):
        lo = c * SH
        rows = np.arange(lo, lo + ROWS) % B
        Ar, Vr = afeat[rows], vfeat[rows]                 # [517, 1024]
        St = np.stack([Ar, Vr], axis=-1)                  # [w, d, s]
        t = np.ascontiguousarray(
            St.reshape(ROWS, NC, 128, 2).transpose(2, 0, 3, 1).reshape(128, TW, NC)
        )
        hn = np.ascontiguousarray(np.stack([Ar[SH:], Vr[SH:]], axis=0))
        in_maps.append({"t": t, "hn": hn})

    res = run_bass_kernel_spmd(nc, in_maps, core_ids=list(range(NCORES)), trace=trace)
    losses = np.concatenate([res.results[c]["loss"] for c in range(NCORES)])

    total = float(np.sum(losses[S:], dtype=np.float64))
    total += sum(_exact_losses_head(vfeat, afeat, range(S)))
    mean = np.float32(total / B)
    return np.asarray(mean, dtype=np.float32), res


def kernel(vfeat, afeat):
    out, _ = run_kernel(vfeat, afeat, trace=False)
    return out


# revision 24
# speedup vs baseline: 1.7208x; 1.0852x over previous
"""Trainium2 Bass kernel for nn_ContrastiveLoss (circular-shift negatives).

Reference computation (B=4096, D=1024, S=5):
    d_p[k]      = ||v[k] - a[k] + eps||
    d_n1[k,m]   = ||v[k] - a[idx(k,m)] + eps||,  idx(k,m) = (k+m+1)%B  (m==k -> (k+1)%B)
    d_n2[k,m]   = ||a[k] - v[idx(k,m)] + eps||
    loss        = mean(relu(1 + 2*d_p - min_m d_n1 - min_m d_n2))

Strategy (8 cores, data-parallel over batch, 512 anchors/core + 5-row halo):
  - All distances via ||x-y||^2 = ||x||^2 + ||y||^2 - 2<x,y> (the +eps term
    is dropped; its effect is ~1e-6 relative, far below tolerance).
  - The host pre-transposes and interleaves the shard into PE-ready layout
    t[p, c, 2w+s] = (A if s==0 else V)[w, 128c+p], so no on-device transposes
    are needed and the DMA loads land directly in matmul operand layout.
  - Loads are split into 4 column-group DMAs sized so that anchor block b's
    band matmuls unblock as soon as group b lands (load/compute pipeline).
  - Per 128-anchor block, two 266-wide f32r band matmuls (1 cycle/row):
    band1 = [V.A^T | V.V^T], band2 = [A.A^T | A.V^T] interleaved windows.
    Bands accumulate in PSUM and are DMA'd straight to a DRAM scratch.
  - One global strided gather (element stride 268 = row pitch + 2) shears the
    11 needed diagonals of every band into lane-aligned SBUF: d_p dot, dn1
    dots, dn2 dots, and the row norms (VV/AA diagonals) per anchor.
  - Shifted norms ||x[k+m+1]||^2 are produced on-chip by 10 tiny shifted-
    identity matmuls (partition shift), with halo-row norms from an ACT
    square+accum over the 5 natural halo rows (tiny extra input).
  - Small vector/scalar/pool epilogue computes sqrt, mins and the hinge.
  - Anchors k<5 (where m==k rewrites the negative index) are recomputed
    exactly on the host in numpy and spliced in; the mean is a host reduce.
"""

import numpy as np

B, D, S = 4096, 1024, 5
NCORES = 8
SH = B // NCORES          # 512 anchors per core
ROWS = SH + S             # 517 rows per shard (incl. halo)
NB = SH // 128            # 4 anchor blocks
NC = D // 128             # 8 contraction chunks
TW = 2 * ROWS             # 1034 interleaved t columns
MARGIN = 1.0
EPS = 1e-6

_CACHE = {}


def _build():
    import concourse.bass as bass
    import concourse.bacc as bacc
    import concourse.tile as tile
    import concourse.mybir as mybir

    from contextlib import ExitStack

    f32 = mybir.dt.float32
    f32r = mybir.dt.float32r
    A = mybir.AluOpType

    nc = bacc.Bacc()
    t_ext = nc.declare_dram_parameter("t", [128, TW, NC], f32r, isOutput=False)
    hn_ext = nc.declare_dram_parameter("hn", [2, S, D], f32, isOutput=False)
    loss_ext = nc.declare_dram_parameter("loss", [SH], f32, isOutput=True)

    with tile.TileContext(nc) as tc, ExitStack() as ctx:
        sing = ctx.enter_context(tc.tile_pool(name="sing", bufs=1))
        tp = ctx.enter_context(tc.tile_pool(name="tp", bufs=1))
        ep = ctx.enter_context(tc.tile_pool(name="ep", bufs=1))
        bandp = ctx.enter_context(tc.tile_pool(name="bandp", bufs=3, space="PSUM"))
        bsb = ctx.enter_context(tc.tile_pool(name="bsb", bufs=4))
        shp = ctx.enter_context(tc.tile_pool(name="shp", bufs=1, space="PSUM"))
        dramp = ctx.enter_context(tc.tile_pool(name="dramp", bufs=1, space="DRAM"))

        # preload the ACT Sqrt table early so the epilogue doesn't stall on
        # a ~1.3us LoadActFuncSet.
        warm = sing.tile([128, 1], f32, tag="warm")
        nc.vector.memset(warm[:, :], 1.0)
        nc.scalar.activation(out=warm[:, :], in_=warm[:, :],
                             func=mybir.ActivationFunctionType.Sqrt)

        # partition-shift matrices for the shifted-norm matmuls (see below):
        # Mpack[:, j] = shift-down-by-(j+1): M[i, p] = 1 iff i == p + j + 1
        # Epack[:, j] = wrap rows:           E[i, p] = 1 iff p == 128-(j+1)+i
        ones = sing.tile([128, 128], f32, tag="ones")
        nc.vector.memset(ones[:, :], 1.0)
        mpack = sing.tile([128, S, 128], f32, tag="mpack")
        epack = sing.tile([128, S, 128], f32, tag="epack")
        for j in range(1, S + 1):
            nc.gpsimd.affine_select(out=mpack[:, j - 1, :], in_=ones[:, :],
                                    pattern=[[1, 128]], base=j,
                                    channel_multiplier=-1,
                                    compare_op=A.is_equal, fill=0.0)
            nc.gpsimd.affine_select(out=epack[0:S, j - 1, :], in_=ones[0:S, :],
                                    pattern=[[1, 128]], base=-(128 - j),
                                    channel_multiplier=-1,
                                    compare_op=A.is_equal, fill=0.0)

        # column-major [p, col, chunk] so the 4 group loads cover disjoint
        # flat ranges (Tile's interval dep tracking stays precise).
        tsb = tp.tile([128, TW, NC], f32r, tag="tsb")
        tv = tsb[:, :, :].rearrange("p (w s) c -> p w s c", s=2)  # s: 0=A, 1=V
        hn_sb = tp.tile([128, 2, D], f32, tag="hn")
        # g[p, b, t, j]: gathered diagonals; cols 97/108 hold halo nv/na norms
        # so the shifted-norm matmul reads one affine window (see rhs APs).
        g = ep.tile([128, 120], f32, tag="g")
        gm = g[:, :].rearrange("p (b t m s) -> p b t m s", b=NB + 1, t=2, m=6, s=2)
        scr = ep.tile([128, D], f32, tag="scr")

        # DRAM band scratch: flat(b, t, p, c) = 68096*b + 34048*t + 266*p + c
        band_d = dramp.tile([2 * NB * 128 * 266], f32, tag="band_d")

        # PE p-state warmup: keep the tensor engine continuously busy from
        # t~1.2us so the band matmuls run at full clock when loads land.
        wp = shp.tile([128, 128], f32, tag="wp")
        for _ in range(14):
            nc.tensor.matmul(wp[:, :], ones[:, :], ones[:, :], start=True, stop=True)

        # ---- input loads: 4 column groups; group b unblocks anchor block b
        GRP = [(0, 266), (266, 532), (532, 798), (798, TW)]
        for g0, g1 in GRP:
            nc.sync.dma_start(out=tsb[:, g0:g1, :], in_=t_ext[:, g0:g1, :])
        # tiny natural-layout halo rows (for halo norms only), last: off path
        nc.sync.dma_start(
            out=hn_sb[0:S, :, :],
            in_=bass.AP(tensor=hn_ext, offset=0, ap=[[D, S], [S * D, 2], [1, D]]),
        )

        # halo norms into g's spare cols: nv-halo -> col 97, na-halo -> col 108
        # (the "b=4" slots of the norm window below).
        nc.scalar.activation(out=scr[0:S, :], in_=hn_sb[0:S, 0, :],
                             func=mybir.ActivationFunctionType.Square,
                             accum_out=g[0:S, 108:109])
        nc.scalar.activation(out=scr[0:S, :], in_=hn_sb[0:S, 1, :],
                             func=mybir.ActivationFunctionType.Square,
                             accum_out=g[0:S, 97:98])

        # ---- bands + PSUM->DRAM writes, per block
        for b in range(NB):
            k0 = 128 * b
            bp = bandp.tile([128, 2, 512], f32, tag="bp")
            for c in range(NC):
                nc.tensor.matmul(bp[:, 0, 0:266], tv[:, k0:k0 + 128, 1, c],
                                 tsb[:, 2 * k0:2 * k0 + 266, c],
                                 start=(c == 0), stop=(c == NC - 1))
            for c in range(NC):
                nc.tensor.matmul(bp[:, 1, 0:266], tv[:, k0:k0 + 128, 0, c],
                                 tsb[:, 2 * k0:2 * k0 + 266, c],
                                 start=(c == 0), stop=(c == NC - 1))
            bs = bsb.tile([128, 2, 266], f32, tag="bs")
            if b % 2 == 0:
                nc.vector.tensor_copy(bs[:, :, :], bp[:, :, 0:266])
            else:
                nc.scalar.copy(bs[:, :, :], bp[:, :, 0:266])
            nc.sync.dma_start(
                out=bass.AP(tensor=band_d.tensor, offset=band_d.offset + 68096 * b,
                            ap=[[266, 128], [34048, 2], [1, 266]]),
                in_=bs[:, :, :],
            )
            if b == NB - 2:
                # prefetch diagonals of blocks 0..2 while block 3 is in flight
                nc.sync.dma_start(
                    out=g[:, 0:72],
                    in_=bass.AP(tensor=band_d.tensor, offset=band_d.offset,
                                ap=[[268, 128], [68096, 3], [34048, 2], [1, 12]]),
                )

        # ---- tail gather: block 3's diagonals only
        nc.sync.dma_start(
            out=g[:, 72:96],
            in_=bass.AP(tensor=band_d.tensor, offset=band_d.offset + 68096 * 3,
                        ap=[[268, 128], [34048, 2], [1, 12]]),
        )

        # ---- shifted norms via partition-shift matmuls:
        # psh[p, m, t, b] = norm[t][128*b + p + m + 1]; the rhs windows read
        # the nv/na diagonal columns of g (cols 24b+1 / 24b+12, halo 97/108).
        psh = shp.tile([128, S, 2, NB], f32, tag="psh")
        rhs_main = bass.AP(tensor=g.tensor, offset=g.offset + 1,
                           ap=[[120, 128], [11, 2], [24, NB]])
        for j in range(1, S + 1):
            m = j - 1
            rhs_wrap = bass.AP(tensor=g.tensor, offset=g.offset + 25,
                               ap=[[120, j], [11, 2], [24, NB]])
            nc.tensor.matmul(psh[:, m, :, :], mpack[:, m, :],
                             rhs_main, start=True, stop=False)
            nc.tensor.matmul(psh[:, m, :, :], epack[0:j, m, :],
                             rhs_wrap, start=False, stop=True)
        pv = psh[:, :, :, :].rearrange("p m t b -> p t b m")

        # ---- epilogue
        dn1_dots = gm[:, 0:NB, 0, 1:6, 0]   # <v[k], a[k+m+1]>
        dn2_dots = gm[:, 0:NB, 1, 1:6, 1]   # <a[k], v[k+m+1]>
        dp_dot = gm[:, 0:NB, 0, 0:1, 0]
        nv_col = gm[:, 0:NB, 0, 0:1, 1]
        na_col = gm[:, 0:NB, 1, 0:1, 0]

        base1 = ep.tile([128, NB, S], f32, tag="base1")
        base2 = ep.tile([128, NB, S], f32, tag="base2")
        dsq = ep.tile([128, NB, 11], f32, tag="dsq")
        dall = ep.tile([128, NB, 11], f32, tag="dall")
        dn1m = ep.tile([128, NB], f32, tag="dn1m")
        dn2m = ep.tile([128, NB], f32, tag="dn2m")
        tsum = ep.tile([128, NB], f32, tag="tsum")
        lossn = ep.tile([128, NB], f32, tag="lossn")

        # dn1^2 = -2*dot + (nv[k] + na[k+m+1]);  dn2^2 = -2*dot + (na[k] + nv[k+m+1])
        nc.vector.tensor_add(base1[:, :, :], pv[:, 1, :, :],
                             nv_col.broadcast_to([128, NB, S]))
        nc.vector.scalar_tensor_tensor(out=dsq[:, :, 0:5], in0=dn1_dots, scalar=-2.0,
                                       in1=base1[:, :, :], op0=A.mult, op1=A.add)
        nc.vector.tensor_add(base2[:, :, :], pv[:, 0, :, :],
                             na_col.broadcast_to([128, NB, S]))
        nc.vector.scalar_tensor_tensor(out=dsq[:, :, 5:10], in0=dn2_dots, scalar=-2.0,
                                       in1=base2[:, :, :], op0=A.mult, op1=A.add)
        nc.vector.scalar_tensor_tensor(out=dsq[:, :, 10:11], in0=dp_dot, scalar=-2.0,
                                       in1=nv_col, op0=A.mult, op1=A.add)
        nc.vector.tensor_add(dsq[:, :, 10:11], dsq[:, :, 10:11], na_col)

        nc.scalar.activation(out=dall[:, :, :], in_=dsq[:, :, :],
                             func=mybir.ActivationFunctionType.Sqrt)
        nc.vector.tensor_reduce(out=dn1m[:, :], in_=dall[:, :, 0:5],
                                axis=mybir.AxisListType.X, op=A.min)
        nc.vector.tensor_reduce(out=dn2m[:, :], in_=dall[:, :, 5:10],
                                axis=mybir.AxisListType.X, op=A.min)
        nc.vector.tensor_add(tsum[:, :], dn1m[:, :], dn2m[:, :])
        nc.vector.scalar_tensor_tensor(out=lossn[:, :], in0=dall[:, :, 10], scalar=2.0,
                                       in1=tsum[:, :], op0=A.mult, op1=A.subtract)
        nc.vector.tensor_scalar(out=lossn[:, :], in0=lossn[:, :], scalar1=MARGIN,
                                scalar2=0.0, op0=A.add, op1=A.max)

        nc.sync.dma_start(
            out=bass.AP(tensor=loss_ext, offset=0, ap=[[1, 128], [128, NB]]),
            in_=lossn[:, :],
        )

    nc.finalize()
    return nc


def _exact_losses_head(vfeat, afeat, ks):
    """Exact reference loss for anchors in ks (handles the m==k index rewrite)."""
    v = vfeat.astype(np.float64)
    a = afeat.astype(np.float64)
    out = []
    for k in ks:
        idx = [(m + k + 1) % B if m != k else (k + 1) % B for m in range(S)]
        d_p = np.sqrt(np.sum((v[k] - a[k] + EPS) ** 2))
        d1 = min(np.sqrt(np.sum((v[k] - a[j] + EPS) ** 2)) for j in idx)
        d2 = min(np.sqrt(np.sum((a[k] - v[j] + EPS) ** 2)) for j in idx)
        out.append(max(MARGIN + 2.0 * d_p - d1 - d2, 0.0))
    return out


def run_kernel(vfeat, afeat, trace=False):
    from concourse.bass_utils import run_bass_kernel_spmd

    vfeat = np.ascontiguousarray(np.asarray(vfeat, dtype=np.float32))
    afeat = np.ascontiguousarray(np.asarray(afeat, dtype=np.float32))

    if "nc" not in _CACHE:
        _CACHE["nc"] = _build()
    nc = _CACHE["nc"]

    in_maps = []
    for c in range(NCORES):
        lo = c * SH
        rows = np.arange(lo, lo + ROWS) % B
        Ar, Vr = afeat[rows], vfeat[rows]                 # [517, 1024]
        St = np.stack([Ar, Vr], axis=-1)                  # [w, d, s]
        t = np.ascontiguousarray(
            St.reshape(ROWS, NC, 128, 2).transpose(2, 0, 3, 1).reshape(128, TW, NC)
        )
        hn = np.ascontiguousarray(np.stack([Ar[SH:], Vr[SH:]], axis=0))
        in_maps.append({"t": t, "hn": hn})

    res = run_bass_kernel_spmd(nc, in_maps, core_ids=list(range(NCORES)), trace=trace)
    losses = np.concatenate([res.results[c]["loss"] for c in range(NCORES)])

    total = float(np.sum(losses[S:], dtype=np.float64))
    total += sum(_exact_losses_head(vfeat, afeat, range(S)))
    mean = np.float32(total / B)
    return np.asarray(mean, dtype=np.float32), res


def kernel(vfeat, afeat):
    out, _ = run_kernel(vfeat, afeat, trace=False)
    return out
